# revision 7
# baseline (speedup 1.0000x reference)
"""Trainium2 Bass kernel for nn_DeepGraphTransformer_mse (8 NeuronCores).

Strategy (edge-cut graph parallelism):
  - 25k active nodes (all edges live among them) are partitioned into 8
    slabs of 3200 (degree-sorted desc per core, padded); the 25k edge-free
    nodes are likewise slabbed. Weights are replicated.
  - Per layer each core computes K'=k+A, V'=v+A for its slab (LN/QKV
    matmuls in fp32r at full PE rate, biases via a ones-row K=1 matmul),
    AllGathers the bf16 [25600,512] K'V' table, then processes its own
    dst nodes: batched dma_gather of K'V' rows per CSR slot, dense
    per-slot softmax (mask from host, s2 = q.B' folded per node), weighted
    aggregation, Wo + gated residual, and a fused FF (W1 in transposed
    layout so W2 needs no transposes; gelu+b1 fused on ACT).
  - Gate logits and the final z projection are folded into spare matmul
    columns (sigmoid(col + bias) on ACT), so gating costs ~3 vector ops.
"""
import os
import sys

sys.path.insert(0, "/opt/trn_rl_repo")

import numpy as np
import ml_dtypes

import concourse.bass as bass
import concourse.bacc as bacc
import concourse.tile as tile
from concourse import mybir
from concourse.bass_utils import run_bass_kernel_spmd
from concourse.masks import make_identity

F32 = mybir.dt.float32
F32R = mybir.dt.float32r
BF16 = mybir.dt.bfloat16
I16 = mybir.dt.int16
AF = mybir.ActivationFunctionType
OP = mybir.AluOpType

DIM, H, DH, FFD, DEPTH = 256, 4, 64, 1024, 2
N, E, HALF = 50000, 100000, 25000
C = 8
SLAB = 3200
TILES_A = SLAB // 128  # 25
G_MAX = 14             # max gather groups per chunk

LAST_RESULT = None  # BassKernelResults of the most recent run (for test.py)


# ----------------------------------------------------------------------------
# host-side preparation (validated against reference in decomp_check.py)
# ----------------------------------------------------------------------------
def _host_prep(inputs):
    x = np.asarray(inputs["x"], np.float32)
    ei = np.asarray(inputs["edge_index"])
    src_g, dst_g = ei[0].astype(np.int64), ei[1].astype(np.int64)
    deg = np.bincount(dst_g, minlength=HALF).astype(np.int64)

    perms = []
    agrow = np.full(HALF, -1, np.int64)
    for c in range(C):
        lo, hi = 3200 * c, min(3200 * (c + 1), HALF)
        g = np.arange(lo, hi)
        g = g[np.argsort(-deg[g], kind="stable")]
        perm = np.concatenate([g, np.full(SLAB - len(g), -1, np.int64)])
        perms.append(perm)
        agrow[g] = 3200 * c + np.arange(len(g))

    dsched = np.zeros(TILES_A, np.int64)
    for c in range(C):
        dg = np.where(perms[c] >= 0, deg[np.maximum(perms[c], 0)], 0)
        dsched = np.maximum(dsched, dg.reshape(TILES_A, 128).max(1))
    dmax = int(dsched.max())

    order = np.argsort(dst_g, kind="stable")
    ss, ds = src_g[order], dst_g[order]
    starts = np.searchsorted(ds, np.arange(HALF))
    ends = np.searchsorted(ds, np.arange(HALF) + 1)

    src_idx = np.zeros((C, TILES_A, dmax, 128), np.int64)
    mask = np.full((C, TILES_A, dmax, 128), -1e30, np.float32)
    for c in range(C):
        for t in range(TILES_A):
            for p in range(128):
                g = perms[c][t * 128 + p]
                if g < 0:
                    continue
                sl = ss[starts[g]:ends[g]]
                src_idx[c, t, :len(sl), p] = agrow[sl]
                mask[c, t, :len(sl), p] = 0.0

    W = {k: np.asarray(v, np.float32) for k, v in inputs.items()}
    scale = DH ** -0.5
    P = dict(perms=perms, dsched=dsched, src_idx=src_idx, mask=mask,
             wq=[], wkvab=[], wo=[], w1=[], b1c=[], w2=[], smallw=[],
             gatev=[], consts=[])
    for l in range(DEPTH):
        g1, g2, g3 = (W["g_attn"][l][i * 256:(i + 1) * 256, 0] for i in range(3))
        Ua, Wa = g1 + g3, g2 - g3
        f1, f2, f3 = (W["g_ff"][l][i * 256:(i + 1) * 256, 0] for i in range(3))
        Uf, Wf = f1 + f3, f2 - f3
        wout = W["Wout"][:, 0]

        m = np.zeros((257, 258), np.float32)
        m[0:256, 0:256] = (W["ln1_g"][l][:, None] * W["Wq"][l]) * scale
        m[256, 0:256] = (W["bq"][l] + W["ln1_b"][l] @ W["Wq"][l]) * scale
        m[0:256, 256] = Wa
        P["wq"].append(m)

        m = np.zeros((513, 512), np.float32)
        m[0:256, :] = W["ln1_g"][l][:, None] * W["Wkv"][l]
        WA = W["We"][l][0:256, :]
        m[256:512, 0:256] = WA
        m[256:512, 256:512] = WA
        m[512, :] = W["bkv"][l] + W["ln1_b"][l] @ W["Wkv"][l]
        P["wkvab"].append(m)

        m = np.zeros((257, 258), np.float32)
        m[0:256, 0:256] = W["Wo"][l]
        m[256, 0:256] = W["bo"][l]
        m[0:256, 256] = W["Wo"][l] @ Ua
        m[256, 256] = W["bo"][l] @ Ua
        P["wo"].append(m)

        W1_eff = W["ln2_g"][l][:, None] * W["W1"][l]
        b1_eff = W["b1"][l] + W["ln2_b"][l] @ W["W1"][l]
        P["w1"].append(W1_eff)
        P["b1c"].append(b1_eff.reshape(8, 128).T.copy())

        m = np.zeros((1025, 258), np.float32)
        m[0:1024, 0:256] = W["W2"][l]
        m[0:1024, 256] = W["W2"][l] @ Uf
        m[0:1024, 257] = W["W2"][l] @ wout
        m[1024, 0:256] = W["b2"][l]
        m[1024, 256] = W["b2"][l] @ Uf
        m[1024, 257] = W["b2"][l] @ wout
        P["w2"].append(m)

        m = np.zeros((256, 2), np.float32)
        m[:, 0] = Wf
        m[:, 1] = wout
        P["smallw"].append(m)

        P["gatev"].append(np.stack([Wa, W["bo"][l]]))
        P["consts"].append(dict(
            sum_Wa=float(Wa.sum()), bo_u=float(W["bo"][l] @ Ua),
            sum_Wf=float(Wf.sum()), sum_wout=float(wout.sum()),
            bout=float(np.asarray(inputs["bout"]).reshape(-1)[0])))

    P["wbb"] = np.concatenate([
        np.concatenate([W["We"][l][256:512, :] for l in range(DEPTH)], 1),
        np.concatenate([W["be"][l][None, :] for l in range(DEPTH)], 1)], 0)

    x_slabs = []
    for c in range(C):
        xa = np.zeros((SLAB, DIM), np.float32)
        real = perms[c] >= 0
        xa[real] = x[perms[c][real]]
        ilo, ihi = HALF + 3200 * c, min(HALF + 3200 * (c + 1), N)
        xi = np.zeros((SLAB, DIM), np.float32)
        xi[:ihi - ilo] = x[ilo:ihi]
        x_slabs.append(np.ascontiguousarray(np.concatenate([xa, xi], 0)))
    P["x_slabs"] = x_slabs

    # gather chunks: greedy over tiles, sum of d <= G_MAX (edge tiles only)
    chunks, cur, curg = [], [], 0
    for t in range(TILES_A):
        d = int(dsched[t])
        if d == 0:
            continue
        if curg + d > G_MAX and cur:
            chunks.append(cur)
            cur, curg = [], 0
        cur.append(t)
        curg += d
    if cur:
        chunks.append(cur)
    P["chunks"] = chunks

    goff = np.zeros(TILES_A + 1, np.int64)
    for t in range(TILES_A):
        goff[t + 1] = goff[t] + dsched[t]
    P["goff"] = goff
    gtot = int(goff[-1])

    # gather runs: each chunk split into runs of <= GRUN groups (HW limit:
    # dma_gather fails above ~1024 indices per call)
    GRUN = 8
    runs = []   # list per chunk: list of (goff_in_chunk, glen)
    for ch in chunks:
        gch = int(sum(dsched[t] for t in ch))
        r, g0 = [], 0
        while g0 < gch:
            gl = min(GRUN, gch - g0)
            r.append((g0, gl))
            g0 += gl
        runs.append(r)
    P["runs"] = runs

    # per-core packed idx (int16, wrapped per run + replicated) and mask
    idx_cols = sum(gl * 8 for r in runs for (_, gl) in r)
    idx_in = np.zeros((C, 128, idx_cols), np.int16)
    mask_in = np.zeros((C, 128, gtot), np.float32)
    for c in range(C):
        col = 0
        for ci, ch in enumerate(chunks):
            flat = []
            for t in ch:
                for j in range(int(dsched[t])):
                    flat.append(src_idx[c, t, j, :])
            flat = np.concatenate(flat).astype(np.int16)  # [Gch*128]
            for (g0, gl) in runs[ci]:
                seg = flat[g0 * 128:(g0 + gl) * 128]
                ncols = gl * 8
                wrp = np.zeros((16, ncols), np.int16)
                ii = np.arange(len(seg))
                wrp[ii % 16, ii // 16] = seg
                idx_in[c, :, col:col + ncols] = np.tile(wrp, (8, 1))
                col += ncols
        for t in range(TILES_A):
            for j in range(int(dsched[t])):
                mask_in[c, :, goff[t] + j] = mask[c, t, j, :]
    P["idx_in"], P["mask_in"], P["gtot"], P["idx_cols"] = idx_in, mask_in, gtot, idx_cols
    return P


# ----------------------------------------------------------------------------
# device program
# ----------------------------------------------------------------------------
def _build(P):
    dsched = P["dsched"]
    chunks = P["chunks"]
    goff = P["goff"]
    gtot = P["gtot"]
    idx_cols = P["idx_cols"]
    dmax = int(dsched.max())
    consts = P["consts"]

    nc = bacc.Bacc("TRN2", target_bir_lowering=False, debug=False, num_devices=C)

    # ---- I/O ----
    x_d = nc.dram_tensor("x_slab", [2 * SLAB, DIM], F32, kind="ExternalInput")
    idx_d = nc.dram_tensor("idx_in", [128, idx_cols], I16, kind="ExternalInput")
    mask_d = nc.dram_tensor("mask_in", [128, gtot], F32, kind="ExternalInput")
    wq_d = nc.dram_tensor("wq", [DEPTH, 257, 258], F32, kind="ExternalInput")
    wkvab_d = nc.dram_tensor("wkvab", [DEPTH, 513, 512], F32, kind="ExternalInput")
    wbb_d = nc.dram_tensor("wbb", [257, 512], F32, kind="ExternalInput")
    wo_d = nc.dram_tensor("wo", [DEPTH, 257, 258], F32, kind="ExternalInput")
    w1_d = nc.dram_tensor("w1", [DEPTH, 256, 1024], F32, kind="ExternalInput")
    w2_d = nc.dram_tensor("w2", [DEPTH, 1025, 258], F32, kind="ExternalInput")
    smallw_d = nc.dram_tensor("smallw", [DEPTH, 256, 2], F32, kind="ExternalInput")
    b1c_d = nc.dram_tensor("b1c", [DEPTH, 128, 8], F32, kind="ExternalInput")
    gatev_d = nc.dram_tensor("gatev", [DEPTH, 2, 256], F32, kind="ExternalInput")
    z_d = nc.dram_tensor("z_out", [2 * SLAB, 1], F32, kind="ExternalOutput")

    with tile.TileContext(nc) as tc:
        with tc.tile_pool(name="wp", bufs=1) as wp, \
             tc.tile_pool(name="sp", bufs=6) as sp, \
             tc.tile_pool(name="st", bufs=2) as stp, \
             tc.tile_pool(name="bp", bufs=2) as bpool, \
             tc.tile_pool(name="pa", bufs=3, space="PSUM") as ps_a, \
             tc.tile_pool(name="ptr", bufs=2, space="PSUM") as ps_tr, \
             tc.tile_pool(name="ph", bufs=1, space="PSUM") as ps_h, \
             tc.tile_pool(name="dr", bufs=1, space="DRAM") as dr:

            # ---------------- constants / weights into SBUF ----------------
            ident = wp.tile([128, 128], F32, name="ident")
            make_identity(nc, ident[:])
            ident_r = wp.tile([128, 128], F32R, name="ident_r")
            nc.vector.tensor_copy(out=ident_r[:], in_=ident[:])
            ident_bf = wp.tile([128, 128], BF16, name="ident_bf")
            nc.vector.tensor_copy(out=ident_bf[:], in_=ident[:])
            ones_f = wp.tile([128, 128], F32, name="ones_f")
            nc.vector.memset(ones_f[:], 1.0)
            ones_r = wp.tile([128, 128], F32R, name="ones_r")
            nc.vector.tensor_copy(out=ones_r[:], in_=ones_f[:])
            ones_bf = wp.tile([128, 128], BF16, name="ones_bf")
            nc.vector.tensor_copy(out=ones_bf[:], in_=ones_f[:])
            eps_t = wp.tile([128, 1], F32, name="eps_t")
            nc.vector.memset(eps_t[:], 1e-5)
            bout_t = wp.tile([128, 1], F32, name="bout_t")
            nc.vector.memset(bout_t[:], consts[DEPTH - 1]["bout"])
            bo_u_t = []
            for l in range(DEPTH):
                bt = wp.tile([128, 1], F32, name=f"bo_u_{l}")
                nc.vector.memset(bt[:], consts[l]["bo_u"])
                bo_u_t.append(bt)

            def load_rt(dram_ap, shape, name, dt=F32R):
                stage = stp.tile([128, 512], F32, name="wstage", tag="wstage")
                nc.sync.dma_start(out=stage[:shape[0], :shape[1]], in_=dram_ap)
                t = wp.tile([128, shape[1]], dt, name=name)
                nc.vector.tensor_copy(out=t[:shape[0], :], in_=stage[:shape[0], :shape[1]])
                return t

            # bias rows live in packed tiles (partition 0)
            bias_r = wp.tile([128, 2 * 258 + 2 * 512 + 512 + 2 * 258], F32R, name="bias_r")
            bias_bf = wp.tile([128, 2 * 258], BF16, name="bias_bf")

            def load_bias(dram_ap, ncols, dst_tile, off, dt):
                stage = stp.tile([128, 512], F32, name="bstage", tag="wstage")
                nc.sync.dma_start(out=stage[0:1, :ncols], in_=dram_ap)
                nc.vector.tensor_copy(out=dst_tile[0:1, off:off + ncols],
                                      in_=stage[0:1, :ncols])

            wq_t, wkvab_t, wo_t, w1_t, w2_t, small_t = [], [], [], [], [], []
            off_q, off_kv, off_wo, off_w2 = [], [], [], []
            OBB = 2 * 258 + 2 * 512
            boff = 0
            for l in range(DEPTH):
                wq_t.append([load_rt(wq_d[l, kt * 128:(kt + 1) * 128, :], (128, 258),
                                     f"wq_{l}_{kt}") for kt in range(2)])
                off_q.append(boff)
                load_bias(wq_d[l, 256:257, :], 258, bias_r, boff, F32R)
                boff += 258
                wkvab_t.append([load_rt(wkvab_d[l, kt * 128:(kt + 1) * 128, :], (128, 512),
                                        f"wkvab_{l}_{kt}") for kt in range(4)])
                off_kv.append(boff)
                load_bias(wkvab_d[l, 512:513, :], 512, bias_r, boff, F32R)
                boff += 512
                wo_t.append([load_rt(wo_d[l, kt * 128:(kt + 1) * 128, :], (128, 258),
                                     f"wo_{l}_{kt}") for kt in range(2)])
                off_wo.append(boff)
                load_bias(wo_d[l, 256:257, :], 258, bias_r, boff, F32R)
                boff += 258
                w1_t.append([[load_rt(w1_d[l, kt * 128:(kt + 1) * 128, mc * 128:(mc + 1) * 128],
                                      (128, 128), f"w1_{l}_{kt}_{mc}", BF16)
                              for mc in range(8)] for kt in range(2)])
                w2_t.append([load_rt(w2_d[l, kt * 128:(kt + 1) * 128, :], (128, 258),
                                     f"w2_{l}_{kt}", BF16) for kt in range(8)])
                off_w2.append(258 * l)
                load_bias(w2_d[l, 1024:1025, :], 258, bias_bf, 258 * l, BF16)
                small_t.append(load_rt(smallw_d[l, 0:128, :], (128, 2),
                                       f"small_a_{l}", BF16))
            # smallw needs both K halves: load as two tiles per layer
            small2_t = []
            for l in range(DEPTH):
                small2_t.append(load_rt(smallw_d[l, 128:256, :], (128, 2),
                                        f"small_b_{l}", BF16))
            wbb_t = [load_rt(wbb_d[kt * 128:(kt + 1) * 128, :], (128, 512),
                             f"wbb_{kt}") for kt in range(2)]
            off_bb = boff
            load_bias(wbb_d[256:257, :], 512, bias_r, boff, F32R)
            boff += 512

            b1c_t = []
            for l in range(DEPTH):
                bt = wp.tile([128, 8], F32, name=f"b1c_{l}")
                nc.sync.dma_start(out=bt[:], in_=b1c_d[l])
                b1c_t.append(bt)
            wa_bc, bo_bc = [], []
            for l in range(DEPTH):
                t1 = wp.tile([128, 256], F32, name=f"wa_bc_{l}")
                ap = gatev_d[l, 0:1, :]
                nc.sync.dma_start(out=t1[:], in_=bass.AP(
                    tensor=ap.tensor, offset=ap.offset, ap=[[0, 128], [1, 256]]))
                wa_bc.append(t1)
                t2 = wp.tile([128, 256], F32, name=f"bo_bc_{l}")
                ap = gatev_d[l, 1:2, :]
                nc.sync.dma_start(out=t2[:], in_=bass.AP(
                    tensor=ap.tensor, offset=ap.offset, ap=[[0, 128], [1, 256]]))
                bo_bc.append(t2)

            idx_sb = wp.tile([128, idx_cols], I16, name="idx_sb")
            nc.sync.dma_start(out=idx_sb[:], in_=idx_d[:])
            mask_sb = wp.tile([128, gtot], F32, name="mask_sb")
            nc.sync.dma_start(out=mask_sb[:], in_=mask_d[:])

            # small persistent slabs (big ones live in DRAM, reloaded per tile)
            s2_slab = wp.tile([128, TILES_A, 4], F32, name="s2_slab")
            lva_slab = wp.tile([128, TILES_A], F32, name="lva_slab")

            # DRAM scratch
            kv_local = [dr.tile([SLAB, 512], BF16, name=f"kv_local_{l}") for l in range(DEPTH)]
            kv_full = [dr.tile([C * SLAB, 512], BF16, addr_space="Shared",
                               name=f"kv_full_{l}") for l in range(DEPTH)]
            nodes_scr = dr.tile([2 * SLAB, DIM], F32, name="nodes_scr")
            xT_dram = dr.tile([SLAB, 256], F32R, name="xT_dram")
            b_dram = [dr.tile([SLAB, 256], BF16, name=f"b_dram_{l}") for l in range(DEPTH)]
            q_dram = dr.tile([SLAB, 256], BF16, name="q_dram")

            # ---------------- helpers ----------------
            def mk_ap(base_ap, aps):
                return bass.AP(tensor=base_ap.tensor, offset=base_ap.offset, ap=aps)

            def layer_norm(src_ap):
                """returns (mv, std, rstd) tiles"""
                stats = sp.tile([128, nc.vector.BN_STATS_DIM], F32, name="stats", tag="stats")
                nc.vector.bn_stats(out=stats[:], in_=src_ap)
                mv = sp.tile([128, nc.vector.BN_AGGR_DIM], F32, name="mv", tag="mv")
                nc.vector.bn_aggr(out=mv[:], in_=stats[:])
                std = sp.tile([128, 1], F32, name="std", tag="std")
                nc.scalar.activation(out=std[:], in_=mv[:, 1:2], func=AF.Sqrt,
                                     bias=eps_t[:], scale=1.0)
                rstd = sp.tile([128, 1], F32, name="rstd", tag="rstd")
                nc.vector.reciprocal(out=rstd[:], in_=std[:])
                return mv, std, rstd

            def transpose2(src_tile, dst_tile, idn, dt=F32):
                for hh in range(2):
                    tp = ps_tr.tile([128, 128], dt, name=f"tp{hh}", tag="tp")
                    nc.tensor.transpose(out=tp[:], in_=src_tile[:, hh * 128:(hh + 1) * 128],
                                        identity=idn[:])
                    nc.vector.tensor_copy(out=dst_tile[:, hh * 128:(hh + 1) * 128], in_=tp[:])

            def ff_pair(l, mid0, mid1, row0, row1):
                """fused FF + gate for two 128-row tiles (node-major inputs)."""
                xn2T = bpool.tile([128, 2, 256], BF16, name="xn2T", tag="xn2T")
                lnout = []
                for i, mid in enumerate((mid0, mid1)):
                    mv2, std2, rstd2 = layer_norm(mid[:])
                    xn2 = bpool.tile([128, 256], BF16, name="xn2", tag="xn2")
                    nc.vector.tensor_scalar(out=xn2[:], in0=mid[:], scalar1=mv2[:, 0:1],
                                            scalar2=rstd2[:], op0=OP.subtract, op1=OP.mult)
                    for hh in range(2):
                        tp = ps_tr.tile([128, 128], BF16, name=f"tpf{hh}", tag="tp")
                        nc.tensor.transpose(out=tp[:], in_=xn2[:, hh * 128:(hh + 1) * 128],
                                            identity=ident_bf[:])
                        nc.vector.tensor_copy(out=xn2T[:, hh, i * 128:(i + 1) * 128], in_=tp[:])
                    lnout.append((mv2, std2))
                # W1 (layout b): h_T chunks [feat128, 256 nodes]
                h_bf = bpool.tile([128, 8, 256], BF16, name="h_bf", tag="h_bf", bufs=2)
                for grp in range(2):
                    ph = ps_h.tile([128, 4, 256], F32, name=f"ph{grp}", tag="ph")
                    for mcl in range(4):
                        mc = grp * 4 + mcl
                        for kt in range(2):
                            nc.tensor.matmul(out=ph[:, mcl, :], lhsT=w1_t[l][kt][mc][:],
                                             rhs=xn2T[:, kt, :], start=(kt == 0), stop=(kt == 1))
                    for mcl in range(4):
                        mc = grp * 4 + mcl
                        nc.scalar.activation(out=h_bf[:, mc, :], in_=ph[:, mcl, :],
                                             func=AF.Gelu, bias=b1c_t[l][:, mc:mc + 1], scale=1.0)
                # small matmul xn2 @ [Wf | wout] per half
                for i, (mid, row) in enumerate(((mid0, row0), (mid1, row1))):
                    psm = ps_tr.tile([128, 128], F32, name="psm", tag="tp")
                    nc.tensor.matmul(out=psm[:, 0:2], lhsT=xn2T[:, 0, i * 128:(i + 1) * 128],
                                     rhs=small_t[l][:, 0:2], start=True, stop=False)
                    nc.tensor.matmul(out=psm[:, 0:2], lhsT=xn2T[:, 1, i * 128:(i + 1) * 128],
                                     rhs=small2_t[l][:, 0:2], start=False, stop=True)
                    pf = ps_a.tile([128, 512], F32, name="pf", tag="pa")
                    for kt in range(8):
                        nc.tensor.matmul(out=pf[:, 0:258], lhsT=h_bf[:, kt, i * 128:(i + 1) * 128],
                                         rhs=w2_t[l][kt][:], start=(kt == 0), stop=False)
                    nc.tensor.matmul(out=pf[:, 0:258], lhsT=ones_bf[0:1, :],
                                     rhs=bias_bf[0:1, off_w2[l]:off_w2[l] + 258],
                                     start=False, stop=True)
                    mv2, std2 = lnout[i]
                    mu_f = sp.tile([128, 1], F32, name="mu_f", tag="mu_f")
                    nc.scalar.mul(out=mu_f[:], in_=mv2[:, 0:1], mul=consts[l]["sum_Wf"])
                    logit_f = sp.tile([128, 1], F32, name="logit_f", tag="logit_f")
                    nc.vector.tensor_scalar(out=logit_f[:], in0=psm[:, 0:1], scalar1=std2[:],
                                            scalar2=mu_f[:], op0=OP.mult, op1=OP.add)
                    gate_f = sp.tile([128, 1], F32, name="gate_f", tag="gate_f")
                    nc.scalar.activation(out=gate_f[:], in_=pf[:, 256:257], func=AF.Sigmoid,
                                         bias=logit_f[:], scale=1.0)
                    mid_t = mid0 if i == 0 else mid1
                    if l < DEPTH - 1:
                        dff = bpool.tile([128, 256], F32, name="dff", tag="dff")
                        nc.vector.tensor_tensor(out=dff[:], in0=pf[:, 0:256], in1=mid_t[:],
                                                op=OP.subtract)
                        nxt = bpool.tile([128, 256], F32, name="nxt", tag="nxt")
                        nc.vector.scalar_tensor_tensor(out=nxt[:], in0=dff[:], scalar=gate_f[:],
                                                       in1=mid_t[:], op0=OP.mult, op1=OP.add)
                        nc.sync.dma_start(out=nodes_scr[row:row + 128, :], in_=nxt[:])
                    else:
                        mu_z = sp.tile([128, 1], F32, name="mu_z", tag="mu_z")
                        nc.scalar.mul(out=mu_z[:], in_=mv2[:, 0:1], mul=consts[l]["sum_wout"])
                        zres = sp.tile([128, 1], F32, name="zres", tag="zres")
                        nc.vector.tensor_scalar(out=zres[:], in0=psm[:, 1:2], scalar1=std2[:],
                                                scalar2=mu_z[:], op0=OP.mult, op1=OP.add)
                        zresb = sp.tile([128, 1], F32, name="zresb", tag="zresb")
                        nc.scalar.activation(out=zresb[:], in_=zres[:], func=AF.Identity,
                                             bias=bout_t[:], scale=1.0)
                        zdiff = sp.tile([128, 1], F32, name="zdiff", tag="zdiff")
                        nc.vector.tensor_tensor(out=zdiff[:], in0=pf[:, 257:258], in1=zresb[:],
                                                op=OP.subtract)
                        zt = sp.tile([128, 1], F32, name="zt", tag="zt")
                        nc.vector.scalar_tensor_tensor(out=zt[:], in0=zdiff[:], scalar=gate_f[:],
                                                       in1=zresb[:], op0=OP.mult, op1=OP.add)
                        nc.sync.dma_start(out=z_d[row:row + 128, :], in_=zt[:])

            def bo_mid(l, nodes_t):
                """gated residual with attn_out == bo (edge-free nodes)."""
                trash = bpool.tile([128, 256], F32, name="trash", tag="trash")
                logit = sp.tile([128, 1], F32, name="logit_i", tag="logit_i")
                nc.vector.scalar_tensor_tensor(out=trash[:], in0=nodes_t[:], scalar=1.0,
                                               in1=wa_bc[l][:], op0=OP.mult, op1=OP.mult,
                                               accum_out=logit[:])
                gate = sp.tile([128, 1], F32, name="gate_i", tag="gate_i")
                nc.scalar.activation(out=gate[:], in_=logit[:], func=AF.Sigmoid,
                                     bias=bo_u_t[l][:], scale=1.0)
                dif = bpool.tile([128, 256], F32, name="dif_i", tag="dif_i")
                nc.vector.tensor_tensor(out=dif[:], in0=bo_bc[l][:], in1=nodes_t[:],
                                        op=OP.subtract)
                mid = bpool.tile([128, 256], F32, name="mid_i", tag="mid_i")
                nc.vector.scalar_tensor_tensor(out=mid[:], in0=dif[:], scalar=gate[:],
                                               in1=nodes_t[:], op0=OP.mult, op1=OP.add)
                return mid

            # ---------------- phase A: xT slab + B' slabs ----------------
            for t in range(TILES_A):
                xa = bpool.tile([128, 256], F32, name="xa", tag="xa")
                nc.sync.dma_start(out=xa[:], in_=x_d[t * 128:(t + 1) * 128, :])
                xTt = bpool.tile([128, 256], F32R, name="xTt", tag="xTt")
                transpose2(xa, xTt, ident)
                nc.sync.dma_start(out=xT_dram[t * 128:(t + 1) * 128, :], in_=xTt[:])
                pb = ps_a.tile([128, 512], F32, name="pb", tag="pa")
                for kt in range(2):
                    nc.tensor.matmul(out=pb[:], lhsT=xTt[:, kt * 128:(kt + 1) * 128],
                                     rhs=wbb_t[kt][:], start=(kt == 0), stop=False)
                nc.tensor.matmul(out=pb[:], lhsT=ones_r[0:1, :],
                                 rhs=bias_r[0:1, off_bb:off_bb + 512], start=False, stop=True)
                for l in range(DEPTH):
                    bpt0 = bpool.tile([128, 256], BF16, name="bpt0", tag="bpt0")
                    nc.vector.tensor_copy(out=bpt0[:], in_=pb[:, l * 256:(l + 1) * 256])
                    nc.sync.dma_start(out=b_dram[l][t * 128:(t + 1) * 128, :], in_=bpt0[:])

            # ---------------- layers ----------------
            KSTOP = int(os.environ.get("KSTOP", "5"))
            for l in range(DEPTH):
                if KSTOP < 2:
                    break
                src_nodes = x_d if l == 0 else nodes_scr
                # ---- phase B ----
                for t in range(TILES_A):
                    nt = bpool.tile([128, 256], F32, name="ntB", tag="ntB")
                    nc.sync.dma_start(out=nt[:], in_=src_nodes[t * 128:(t + 1) * 128, :])
                    mv, std, rstd = layer_norm(nt[:])
                    xn = bpool.tile([128, 256], F32R, name="xn", tag="xn")
                    nc.vector.tensor_scalar(out=xn[:], in0=nt[:], scalar1=mv[:, 0:1],
                                            scalar2=rstd[:], op0=OP.subtract, op1=OP.mult)
                    xnT = bpool.tile([128, 256], F32R, name="xnT", tag="xnT")
                    transpose2(xn, xnT, ident_r, F32R)
                    pq = ps_a.tile([128, 512], F32, name="pq", tag="pa")
                    for kt in range(2):
                        nc.tensor.matmul(out=pq[:, 0:258], lhsT=xnT[:, kt * 128:(kt + 1) * 128],
                                         rhs=wq_t[l][kt][:], start=(kt == 0), stop=False)
                    nc.tensor.matmul(out=pq[:, 0:258], lhsT=ones_r[0:1, :],
                                     rhs=bias_r[0:1, off_q[l]:off_q[l] + 258],
                                     start=False, stop=True)
                    xTt2 = bpool.tile([128, 256], F32R, name="xTt2", tag="xTt")
                    nc.sync.dma_start(out=xTt2[:], in_=xT_dram[t * 128:(t + 1) * 128, :])
                    pkv = ps_a.tile([128, 512], F32, name="pkv", tag="pa")
                    lhs4 = [xnT[:, 0:128], xnT[:, 128:256],
                            xTt2[:, 0:128], xTt2[:, 128:256]]
                    for kt in range(4):
                        nc.tensor.matmul(out=pkv[:], lhsT=lhs4[kt], rhs=wkvab_t[l][kt][:],
                                         start=(kt == 0), stop=False)
                    nc.tensor.matmul(out=pkv[:], lhsT=ones_r[0:1, :],
                                     rhs=bias_r[0:1, off_kv[l]:off_kv[l] + 512],
                                     start=False, stop=True)
                    kvbf = bpool.tile([128, 512], BF16, name="kvbf", tag="kvbf")
                    nc.vector.tensor_copy(out=kvbf[:], in_=pkv[:])
                    nc.sync.dma_start(out=kv_local[l][t * 128:(t + 1) * 128, :], in_=kvbf[:])
                    qbf = bpool.tile([128, 256], BF16, name="qbf", tag="qbf")
                    nc.vector.tensor_copy(out=qbf[:], in_=pq[:, 0:256])
                    nc.sync.dma_start(out=q_dram[t * 128:(t + 1) * 128, :], in_=qbf[:])
                    bptB = bpool.tile([128, 256], BF16, name="bptB", tag="bptB")
                    nc.sync.dma_start(out=bptB[:], in_=b_dram[l][t * 128:(t + 1) * 128, :])
                    sp2 = bpool.tile([128, 256], BF16, name="sp2", tag="sp2")
                    nc.vector.tensor_tensor(out=sp2[:], in0=qbf[:],
                                            in1=bptB[:], op=OP.mult)
                    nc.vector.tensor_reduce(
                        out=s2_slab[:, t, :], in_=sp2[:].rearrange("p (h f) -> p h f", h=4),
                        axis=mybir.AxisListType.X, op=OP.add)
                    mu_s = sp.tile([128, 1], F32, name="mu_s", tag="mu_s")
                    nc.scalar.mul(out=mu_s[:], in_=mv[:, 0:1], mul=consts[l]["sum_Wa"])
                    nc.vector.tensor_scalar(out=lva_slab[:, t:t + 1], in0=pq[:, 256:257],
                                            scalar1=std[:], scalar2=mu_s[:],
                                            op0=OP.mult, op1=OP.add)

                # ---- AllGather ----
                if KSTOP < 3:
                    continue
                nc.gpsimd.collective_compute(
                    "AllGather", OP.bypass, replica_groups=[list(range(C))],
                    ins=[kv_local[l].opt()], outs=[kv_full[l].opt()])

                # ---- edge chunks + fused output/FF ----
                if KSTOP < 4:
                    continue
                icol = 0
                for ci, ch in enumerate(chunks):
                    gch = int(sum(dsched[t] for t in ch))
                    kvg = bpool.tile([128, G_MAX, 512], BF16, name="kvg", tag="kvg", bufs=2)
                    for (g0, glen) in P["runs"][ci]:
                        nc.gpsimd.dma_gather(kvg[:, g0:g0 + glen, :], kv_full[l][:],
                                             idx_sb[:, icol:icol + glen * 8],
                                             glen * 128, glen * 128, 512)
                        icol += glen * 8
                    KEDGE = int(os.environ.get("KEDGE", "9"))
                    gl = 0
                    for t in ch:
                        d = int(dsched[t])
                        kslice = kvg[:, gl:gl + d, 0:256]
                        vslice = kvg[:, gl:gl + d, 256:512]
                        gl += d
                        qld = bpool.tile([128, 256], BF16, name="qld", tag="qld")
                        nc.sync.dma_start(out=qld[:], in_=q_dram[t * 128:(t + 1) * 128, :])
                        prod = bpool.tile([128, dmax, 256], BF16, name="prod", tag="prod", bufs=2)
                        qa = qld[:]
                        q_bc = mk_ap(qa, [qa.ap[0], [0, d], qa.ap[1]])
                        if KEDGE < 2:
                            continue
                        nc.vector.tensor_tensor(out=prod[:, 0:d, :], in0=kslice, in1=q_bc,
                                                op=OP.mult)
                        sim = bpool.tile([128, 4, dmax], F32, name="sim", tag="sim")
                        sim_out = mk_ap(sim[:], [sim[:].ap[0], [1, d], [dmax, 4]])
                        nc.vector.tensor_reduce(
                            out=sim_out, in_=prod[:, 0:d, :].rearrange("p s (h f) -> p s h f", h=4),
                            axis=mybir.AxisListType.X, op=OP.add)
                        if KEDGE < 3:
                            continue
                        s2a = s2_slab[:, t, :]
                        nc.vector.tensor_tensor(out=sim[:, :, 0:d], in0=sim[:, :, 0:d],
                                                in1=mk_ap(s2a, [s2a.ap[0], [1, 4], [0, d]]),
                                                op=OP.add)
                        ma = mask_sb[:, int(goff[t]):int(goff[t]) + d]
                        nc.vector.tensor_tensor(out=sim[:, :, 0:d], in0=sim[:, :, 0:d],
                                                in1=mk_ap(ma, [ma.ap[0], [0, 4], ma.ap[1]]),
                                                op=OP.add)
                        if KEDGE < 4:
                            continue
                        ez = bpool.tile([128, 4, dmax], BF16, name="ez", tag="ez")
                        nc.scalar.activation(out=ez[:, :, 0:d], in_=sim[:, :, 0:d], func=AF.Exp)
                        den = sp.tile([128, 4], F32, name="den", tag="den")
                        nc.vector.tensor_reduce(out=den[:], in_=ez[:, :, 0:d],
                                                axis=mybir.AxisListType.X, op=OP.add)
                        ind = sp.tile([128, 1], F32, name="ind", tag="ind")
                        nc.vector.tensor_scalar(out=ind[:], in0=den[:, 0:1], scalar1=1e30,
                                                scalar2=1.0, op0=OP.mult, op1=OP.min)
                        nc.vector.tensor_scalar(out=den[:], in0=den[:], scalar1=1e-30,
                                                scalar2=None, op0=OP.add)
                        rec = sp.tile([128, 4], F32, name="rec", tag="rec")
                        nc.vector.reciprocal(out=rec[:], in_=den[:])
                        if KEDGE < 5:
                            continue
                        ea = ez[:]
                        ez_bc = mk_ap(ea, [ea.ap[0], [1, d], [dmax, 4], [0, 64]])
                        nc.vector.tensor_tensor(out=prod[:, 0:d, :], in0=vslice, in1=ez_bc,
                                                op=OP.mult)
                        agg = bpool.tile([128, 256], F32, name="agg", tag="agg")
                        pa0 = prod[:]
                        nc.vector.tensor_reduce(
                            out=agg[:].rearrange("p (h f) -> p h f", h=4),
                            in_=mk_ap(pa0, [pa0.ap[0], [64, 4], [1, 64], [256, d]]),
                            axis=mybir.AxisListType.X, op=OP.add)
                        ra = rec[:]
                        nc.vector.tensor_tensor(
                            out=agg[:].rearrange("p (h f) -> p h f", h=4),
                            in0=agg[:].rearrange("p (h f) -> p h f", h=4),
                            in1=mk_ap(ra, [ra.ap[0], [1, 4], [0, 64]]), op=OP.mult)
                        if KEDGE < 6:
                            continue
                        bptE = bpool.tile([128, 256], BF16, name="bptE", tag="bptE")
                        nc.sync.dma_start(out=bptE[:], in_=b_dram[l][t * 128:(t + 1) * 128, :])
                        bm = bpool.tile([128, 256], F32, name="bm", tag="bm")
                        nc.vector.tensor_scalar_mul(out=bm[:], in0=bptE[:],
                                                    scalar1=ind[:])
                        aggr = bpool.tile([128, 256], F32R, name="aggr", tag="aggr")
                        nc.vector.tensor_tensor(out=aggr[:], in0=agg[:], in1=bm[:], op=OP.add)
                        if KEDGE < 7:
                            continue
                        aggT = bpool.tile([128, 256], F32R, name="aggT", tag="aggT")
                        transpose2(aggr, aggT, ident_r, F32R)
                        po = ps_a.tile([128, 512], F32, name="po", tag="pa")
                        for kt in range(2):
                            nc.tensor.matmul(out=po[:, 0:258],
                                             lhsT=aggT[:, kt * 128:(kt + 1) * 128],
                                             rhs=wo_t[l][kt][:], start=(kt == 0), stop=False)
                        nc.tensor.matmul(out=po[:, 0:258], lhsT=ones_r[0:1, :],
                                         rhs=bias_r[0:1, off_wo[l]:off_wo[l] + 258],
                                         start=False, stop=True)
                        if KEDGE < 8:
                            continue
                        gate = sp.tile([128, 1], F32, name="gate_a", tag="gate_a")
                        nc.scalar.activation(out=gate[:], in_=po[:, 256:257], func=AF.Sigmoid,
                                             bias=lva_slab[:, t:t + 1], scale=1.0)
                        nt2 = bpool.tile([128, 256], F32, name="nt2", tag="nt2")
                        nc.sync.dma_start(out=nt2[:], in_=src_nodes[t * 128:(t + 1) * 128, :])
                        dif = bpool.tile([128, 256], F32, name="dif_a", tag="dif_a")
                        nc.vector.tensor_tensor(out=dif[:], in0=po[:, 0:256], in1=nt2[:],
                                                op=OP.subtract)
                        mid0 = bpool.tile([128, 256], F32, name="mid0", tag="mid0")
                        nc.vector.scalar_tensor_tensor(out=mid0[:], in0=dif[:], scalar=gate[:],
                                                       in1=nt2[:], op0=OP.mult, op1=OP.add)
                        # paired inactive tile
                        nti = bpool.tile([128, 256], F32, name="nti", tag="nti")
                        nc.sync.dma_start(out=nti[:],
                                          in_=src_nodes[SLAB + t * 128:SLAB + (t + 1) * 128, :])
                        mid1 = bo_mid(l, nti)
                        if KSTOP >= 5:
                            ff_pair(l, mid0, mid1, t * 128, SLAB + t * 128)

                # bo-path for active tiles with no edges (d == 0)
                for t in range(TILES_A):
                    if int(dsched[t]) != 0:
                        continue
                    nta = bpool.tile([128, 256], F32, name="nta0", tag="nta0")
                    nc.sync.dma_start(out=nta[:], in_=src_nodes[t * 128:(t + 1) * 128, :])
                    mid0 = bo_mid(l, nta)
                    nti = bpool.tile([128, 256], F32, name="nti0", tag="nti")
                    nc.sync.dma_start(out=nti[:],
                                      in_=src_nodes[SLAB + t * 128:SLAB + (t + 1) * 128, :])
                    mid1 = bo_mid(l, nti)
                    if KSTOP >= 5:
                        ff_pair(l, mid0, mid1, t * 128, SLAB + t * 128)

    return nc


def kernel(**inputs):
    global LAST_RESULT
    P = _host_prep(inputs)
    nc = _build(P)
    nc.finalize()

    shared = dict(
        wq=np.stack(P["wq"]), wkvab=np.stack(P["wkvab"]), wbb=P["wbb"],
        wo=np.stack(P["wo"]), w1=np.stack(P["w1"]), w2=np.stack(P["w2"]),
        smallw=np.stack(P["smallw"]), b1c=np.stack(P["b1c"]),
        gatev=np.stack(P["gatev"]))
    in_maps = []
    for c in range(C):
        m = dict(shared)
        m["x_slab"] = P["x_slabs"][c]
        m["idx_in"] = np.ascontiguousarray(P["idx_in"][c])
        m["mask_in"] = np.ascontiguousarray(P["mask_in"][c])
        in_maps.append(m)

    LAST_RESULT = run_bass_kernel_spmd(
        nc, in_maps, core_ids=list(range(C)),
        trace=bool(int(os.environ.get("KBENCH_TRACE", "0"))))

    z_full = np.zeros((N, 1), np.float32)
    for c in range(C):
        z = LAST_RESULT.results[c]["z_out"]
        n_real = int((P["perms"][c] >= 0).sum())
        z_full[P["perms"][c][:n_real], 0] = z[:n_real, 0]
        ilo, ihi = HALF + 3200 * c, min(HALF + 3200 * (c + 1), N)
        z_full[ilo:ihi, 0] = z[SLAB:SLAB + (ihi - ilo), 0]

    z_old = z_full[:HALF].copy()
    z_new = z_full[HALF:].copy()
    return (z_new.reshape(-1), z_old, z_new)
